# revision 4
# baseline (speedup 1.0000x reference)
"""MoE (mxfp4, top-2 routing) Trainium2 kernel.

Sharding: expert-parallel. 16 experts / 8 cores = 2 experts per core.
Each core computes the dense SwiGLU MLP for its 2 experts over all 128
tokens, scaled by top-2 combine weights (router runs on host). Host sums
the 8 partial outputs (the all-reduce) and adds the combine-weighted
down-bias term (linear in the output, so it commutes with the sum).

Weights are host-decoded from mxfp4 to fp8e5m2 (exact: every mxfp4
value times an e8m0 scale in this problem's range is representable in
e5m2) and streamed as the matmul *moving* operand, so TensorE ingests
them at 1 elem/lane/cycle. Activations stay bf16 (fp8 activations blow
the 2e-2 tolerance; measured 4.2e-2 for e4m3 x alone).

Layout tricks:
- gu weight columns reordered [g0 u0 g1 u1 ...] per 512-chunk so gate/up
  psum pairs consume contiguous column halves.
- h is transposed for the down matmul via fp32-*viewed* PE transposes
  (2 bf16 packed per fp32 lane): 8 transposes per expert instead of 16.
  The down matmul unpacks the pair with stride-2 stationary APs, and the
  down weight layout interleaves f accordingly.
- All weight DMAs are issued up front (12 MB fits in SBUF) on the two
  HWDGE queues so DMA runs at line rate under the compute.
- A few dummy matmuls at t=0 warm the PE HAM clock gate before the
  first weight chunk lands.
"""

import sys
import numpy as np

for _p in ("/opt/trn_rl_repo", "/root/.axon_site/_ro/trn_rl_repo"):
    if _p not in sys.path:
        sys.path.insert(0, _p)

import ml_dtypes

FP4_LUT = np.array(
    [0.0, 0.5, 1.0, 1.5, 2.0, 3.0, 4.0, 6.0,
     -0.0, -0.5, -1.0, -1.5, -2.0, -3.0, -4.0, -6.0],
    dtype=np.float32,
)
BLOCK = 32
E, H, F, T = 16, 1024, 2048, 128
N_CORES = 8
EXP_PER_CORE = E // N_CORES

BF16 = ml_dtypes.bfloat16
E5M2 = ml_dtypes.float8_e5m2

N_WARM = 6  # dummy matmuls to open the HAM clock gate

_compiled = {}


def _dequant(blocks, scales):
    b = blocks.astype(np.uint8)
    lo = b & 0xF
    hi = (b >> 4) & 0xF
    nib = np.stack([lo, hi], axis=-1).reshape(blocks.shape[:-1] + (blocks.shape[-1] * 2,))
    vals = FP4_LUT[nib]
    s = np.exp2(scales.astype(np.float32) - 127.0)
    s = np.repeat(s, BLOCK, axis=-1)
    return vals * s


def _build():
    from concourse import bacc, mybir, tile

    f32 = mybir.dt.float32
    bf16 = mybir.dt.bfloat16
    wdt = mybir.dt.float8e5

    nc = bacc.Bacc("TRN2", target_bir_lowering=False, debug=False,
                   num_devices=N_CORES)

    xTb_d = nc.declare_dram_parameter("xTb", [128, 8, 128], bf16, isOutput=False)
    comb_d = nc.declare_dram_parameter("comb", [128, 2], f32, isOutput=False)
    wg_d = [nc.declare_dram_parameter(f"wg{e}", [128, 8, 4096], wdt, isOutput=False)
            for e in range(EXP_PER_CORE)]
    wd_d = [nc.declare_dram_parameter(f"wd{e}", [128, 16, 1024], wdt, isOutput=False)
            for e in range(EXP_PER_CORE)]
    bgu_d = [nc.declare_dram_parameter(f"bgu{e}", [1, 4096], bf16, isOutput=False)
             for e in range(EXP_PER_CORE)]
    out_d = nc.declare_dram_parameter("out", [128, 1024], f32, isOutput=True)
    dbg_d = nc.declare_dram_parameter("dbg", [128, 128], f32, isOutput=True)

    AF = mybir.ActivationFunctionType
    OP = mybir.AluOpType

    hw_engines = [None, None]  # filled with (sync, scalar) below

    with tile.TileContext(nc) as tc:
        with (
            tc.tile_pool(name="const", bufs=1) as constp,
            tc.tile_pool(name="wg", bufs=16) as wgp,
            tc.tile_pool(name="wd", bufs=8) as wdp,
            tc.tile_pool(name="hp", bufs=2) as hp,
            tc.tile_pool(name="silp", bufs=2) as silp,
            tc.tile_pool(name="htp", bufs=2) as htp,
            tc.tile_pool(name="ytp", bufs=2) as ytp,
            tc.tile_pool(name="psgu", bufs=4, space="PSUM") as ps_gu,
            tc.tile_pool(name="psy", bufs=2, space="PSUM") as ps_yp,
            tc.tile_pool(name="pst", bufs=2, space="PSUM") as ps_tp,
        ):
            dma_engines = [nc.sync, nc.scalar]
            _dma_i = [0]

            def dma(out, in_):
                eng = dma_engines[_dma_i[0] % 2]
                _dma_i[0] += 1
                eng.dma_start(out=out, in_=in_)

            # ---- small constants (HWDGE, land fast) ----
            xT = constp.tile([128, 8, 128], bf16)
            nc.scalar.dma_start(out=xT[:], in_=xTb_d[:])
            combine = constp.tile([128, 2], f32)
            nc.scalar.dma_start(out=combine[:], in_=comb_d[:])
            bgu_t = []
            for e in range(EXP_PER_CORE):
                bg = constp.tile([1, 4096], bf16, tag=f"bgu{e}")
                nc.scalar.dma_start(out=bg[:], in_=bgu_d[e][:])
                bgu_t.append(bg)

            ones_t = constp.tile([1, 128], bf16)
            nc.vector.memset(ones_t[:], 1.0)
            ident = constp.tile([128, 128], f32)
            nc.vector.memset(ident[:], 1.0)
            nc.gpsimd.affine_select(
                out=ident[:], in_=ident[:],
                compare_op=OP.is_equal, fill=0.0, base=0,
                pattern=[[-1, 128]], channel_multiplier=1,
            )

            # ---- all weight DMAs issued up front; everything resident ----
            # gu chunks: [128, 2, 2048] (512 KB): (kp-pair, column half)
            wg_t = [[None] * 8 for _ in range(EXP_PER_CORE)]
            wd_t = [[None] * 4 for _ in range(EXP_PER_CORE)]
            for e in range(EXP_PER_CORE):
                for half in range(2):
                    for kp in range(4):
                        t = wgp.tile([128, 2, 2048], wdt, tag="wg")
                        dma(t[:], wg_d[e][:, 2 * kp:2 * kp + 2,
                                          half * 2048:(half + 1) * 2048])
                        wg_t[e][half * 4 + kp] = t
                for ci in range(4):
                    t = wdp.tile([128, 4, 1024], wdt, tag="wd")
                    dma(t[:], wd_d[e][:, 4 * ci:4 * ci + 4, :])
                    wd_t[e][ci] = t

            # ---- PE warmup: dummy matmuls so HAM is at K=8/8 when the
            # first weight chunk lands. Results funneled to dbg so DCE
            # can't drop them.
            ps_w = ps_gu.tile([128, 512], f32, tag="psgu")
            for i in range(N_WARM):
                nc.tensor.matmul(ps_w[:], ones_t[:], bgu_t[0][:, :512],
                                 start=True, stop=True)
            warm_sb = constp.tile([128, 128], f32, tag="warm")
            nc.scalar.activation(warm_sb[:], ps_w[:, :128], AF.Copy)
            nc.sync.dma_start(out=dbg_d[:], in_=warm_sb[:])

            acc = constp.tile([128, 1024], f32)

            # ---- experts ----
            for e in range(EXP_PER_CORE):
                h_sb = hp.tile([128, 2048], bf16, tag="h")
                hT32 = htp.tile([128, 8, 128], f32, tag="hT")
                for oc in range(4):
                    half, sub = oc // 2, oc % 2
                    ps_g = ps_gu.tile([128, 512], f32, tag="psgu")
                    ps_u = ps_gu.tile([128, 512], f32, tag="psgu")
                    nc.tensor.matmul(
                        ps_g[:], ones_t[:],
                        bgu_t[e][:, oc * 1024:oc * 1024 + 512],
                        start=True, stop=False)
                    nc.tensor.matmul(
                        ps_u[:], ones_t[:],
                        bgu_t[e][:, oc * 1024 + 512:oc * 1024 + 1024],
                        start=True, stop=False)
                    for k in range(8):
                        ch = wg_t[e][half * 4 + k // 2]
                        stat = xT[:, k, :]
                        nc.tensor.matmul(
                            ps_g[:], stat,
                            ch[:, k % 2, sub * 1024:sub * 1024 + 512],
                            start=False, stop=(k == 7))
                        nc.tensor.matmul(
                            ps_u[:], stat,
                            ch[:, k % 2, sub * 1024 + 512:sub * 1024 + 1024],
                            start=False, stop=(k == 7))
                    sil = silp.tile([128, 512], f32, tag="sil")
                    nc.scalar.activation(sil[:], ps_g[:], AF.Silu)
                    nc.vector.tensor_tensor(
                        h_sb[:, oc * 512:(oc + 1) * 512], sil[:], ps_u[:],
                        op=OP.mult)
                    # fp32-viewed transposes of the finished 512 columns
                    for kt in (2 * oc, 2 * oc + 1):
                        ps_t = ps_tp.tile([128, 128], f32, tag="pst")
                        nc.tensor.transpose(
                            ps_t[:],
                            h_sb[:, 256 * kt:256 * (kt + 1)].bitcast(f32),
                            ident[:])
                        nc.vector.tensor_copy(hT32[:, kt, :], ps_t[:])

                # down projection: h.T (bf16 pairs in fp32 lanes) x W_d
                ps_y0 = ps_yp.tile([128, 512], f32, tag="psy")
                ps_y1 = ps_yp.tile([128, 512], f32, tag="psy")
                ps_y = [ps_y0, ps_y1]
                hT16 = hT32[:, :, :].bitcast(bf16)  # [128, 8, 256]
                for kt in range(8):
                    for o in range(2):
                        stat = hT16[:, kt, o::2]
                        ch = wd_t[e][kt // 2]
                        j = 2 * (kt % 2) + o
                        for c in range(2):
                            nc.tensor.matmul(
                                ps_y[c][:], stat,
                                ch[:, j, c * 512:(c + 1) * 512],
                                start=(kt == 0 and o == 0),
                                stop=(kt == 7 and o == 1))
                ce = combine[:, e:e + 1]
                for c in range(2):
                    if e == 0:
                        nc.vector.tensor_scalar(
                            acc[:, c * 512:(c + 1) * 512], ps_y[c][:],
                            ce, None, op0=OP.mult)
                    else:
                        ytmp = ytp.tile([128, 512], f32, tag="ytmp")
                        nc.vector.tensor_scalar(ytmp[:], ps_y[c][:],
                                                ce, None, op0=OP.mult)
                        nc.vector.tensor_tensor(
                            acc[:, c * 512:(c + 1) * 512],
                            acc[:, c * 512:(c + 1) * 512], ytmp[:],
                            op=OP.add)
                        nc.sync.dma_start(
                            out=out_d[:, c * 512:(c + 1) * 512],
                            in_=acc[:, c * 512:(c + 1) * 512])

    nc.finalize()
    return nc


# column permutation: [g0 u0 g1 u1 g2 u2 g3 u3] (512 each)
_COLPERM = np.concatenate(
    [np.r_[oc * 512:(oc + 1) * 512, 2048 + oc * 512:2048 + (oc + 1) * 512]
     for oc in range(4)])

# down f interleave: row j of 16, lane i: f = 256*(j//2) + 2*i + (j%2)
_J = np.arange(16)
_I = np.arange(128)
_DOWN_F = (256 * (_J[:, None] // 2) + 2 * _I[None, :] + (_J[:, None] % 2))  # [16,128]


def _prep_inputs(hidden_states, router_w, bias_gu, bias_down,
                 blocks_gu, scales_gu, blocks_down, scales_down):
    x = np.asarray(hidden_states, dtype=np.float32).reshape(T, H)
    xT = np.ascontiguousarray(x.T)                         # [1024, 128]
    xTb = np.ascontiguousarray(
        xT.reshape(8, 128, 128).transpose(1, 0, 2)).astype(BF16)

    # host router: logits -> top-2 -> softmax -> dense combine [T, E]
    logits = x @ np.asarray(router_w, dtype=np.float32).T
    order = np.argsort(-logits, axis=-1, kind="stable")
    i1, i2 = order[:, 0], order[:, 1]
    v1 = logits[np.arange(T), i1]
    v2 = logits[np.arange(T), i2]
    w1 = 1.0 / (1.0 + np.exp(v2 - v1))
    w2 = 1.0 - w1
    combine = np.zeros((T, E), dtype=np.float32)
    combine[np.arange(T), i1] = w1
    combine[np.arange(T), i2] = w2

    bias_down_f = np.asarray(bias_down, dtype=np.float32)
    host_bias = combine @ bias_down_f                       # [T, H]

    w_gu = _dequant(np.asarray(blocks_gu), np.asarray(scales_gu))      # [E,4096,1024]
    w_dn = _dequant(np.asarray(blocks_down), np.asarray(scales_down))  # [E,1024,2048]
    bias_gu_f = np.asarray(bias_gu, dtype=np.float32)

    in_maps = []
    for core in range(N_CORES):
        my = [core * EXP_PER_CORE + j for j in range(EXP_PER_CORE)]
        m = {"xTb": xTb,
             "comb": np.ascontiguousarray(combine[:, my]).astype(np.float32)}
        for j, ge in enumerate(my):
            wre = w_gu[ge][_COLPERM]                       # [4096, 1024]
            wT = np.ascontiguousarray(wre.T)               # [1024, 4096]
            m[f"wg{j}"] = np.ascontiguousarray(
                wT.reshape(8, 128, 4096).transpose(1, 0, 2)).astype(E5M2)
            # down: wd[i, j2, c] = W_d[c, f(j2, i)]
            wd = w_dn[ge][:, _DOWN_F]                      # [1024, 16, 128]
            m[f"wd{j}"] = np.ascontiguousarray(
                wd.transpose(2, 1, 0)).astype(E5M2)        # [128, 16, 1024]
            m[f"bgu{j}"] = bias_gu_f[ge][_COLPERM].reshape(1, 4096).astype(BF16)
        in_maps.append(m)
    return in_maps, host_bias


def kernel(hidden_states, router_w, bias_gu, bias_down,
           blocks_gu, scales_gu, blocks_down, scales_down, _trace=False):
    from concourse.bass_utils import run_bass_kernel_spmd

    if "nc" not in _compiled:
        _compiled["nc"] = _build()
    nc = _compiled["nc"]

    in_maps, host_bias = _prep_inputs(
        hidden_states, router_w, bias_gu, bias_down,
        blocks_gu, scales_gu, blocks_down, scales_down)
    res = run_bass_kernel_spmd(nc, in_maps, list(range(N_CORES)), trace=_trace)
    total = host_bias.copy()
    for om in res.results:
        total += np.asarray(om["out"], dtype=np.float32)
    out = total.reshape(1, T, H)
    if _trace:
        return out, res
    return out


# revision 10
# speedup vs baseline: 1.1899x; 1.1899x over previous
"""MoE (mxfp4, top-2 routing) Trainium2 kernel.

Sharding: expert-parallel. 16 experts / 8 cores = 2 experts per core.
Each core computes the dense SwiGLU MLP for its 2 experts over all 128
tokens, scaled by top-2 combine weights (router runs on host). Host sums
the 8 partial outputs (the all-reduce) and adds the combine-weighted
down-bias term (linear in the output, so it commutes with the sum).

Weights are host-decoded from mxfp4 to fp8e5m2 (exact: every mxfp4
value times an e8m0 scale in this problem's range is representable in
e5m2) and streamed as the matmul *moving* operand, so TensorE ingests
them at 1 elem/lane/cycle. Activations stay bf16 (fp8 activations blow
the 2e-2 tolerance; measured 4.2e-2 for e4m3 x alone).

Layout tricks:
- gu weight columns reordered [g0 u0 g1 u1 ...] per 512-chunk so gate/up
  psum pairs consume contiguous column halves.
- h is transposed for the down matmul via fp32-*viewed* PE transposes
  (2 bf16 packed per fp32 lane): 8 transposes per expert instead of 16.
  The down matmul unpacks the pair with stride-2 stationary APs, and the
  down weight layout interleaves f accordingly.
- All weight DMAs are issued up front (12 MB fits in SBUF) on the two
  HWDGE queues so DMA runs at line rate under the compute.
- A few dummy matmuls at t=0 warm the PE HAM clock gate before the
  first weight chunk lands.
"""

import sys
import numpy as np

for _p in ("/opt/trn_rl_repo", "/root/.axon_site/_ro/trn_rl_repo"):
    if _p not in sys.path:
        sys.path.insert(0, _p)

import ml_dtypes

FP4_LUT = np.array(
    [0.0, 0.5, 1.0, 1.5, 2.0, 3.0, 4.0, 6.0,
     -0.0, -0.5, -1.0, -1.5, -2.0, -3.0, -4.0, -6.0],
    dtype=np.float32,
)
BLOCK = 32
E, H, F, T = 16, 1024, 2048, 128
N_CORES = 8
EXP_PER_CORE = E // N_CORES

BF16 = ml_dtypes.bfloat16
E5M2 = ml_dtypes.float8_e5m2

N_WARM = 12  # dummy matmuls to open the HAM clock gate

_compiled = {}


def _dequant(blocks, scales):
    b = blocks.astype(np.uint8)
    lo = b & 0xF
    hi = (b >> 4) & 0xF
    nib = np.stack([lo, hi], axis=-1).reshape(blocks.shape[:-1] + (blocks.shape[-1] * 2,))
    vals = FP4_LUT[nib]
    s = np.exp2(scales.astype(np.float32) - 127.0)
    s = np.repeat(s, BLOCK, axis=-1)
    return vals * s


def _build():
    from concourse import bacc, mybir, tile

    f32 = mybir.dt.float32
    bf16 = mybir.dt.bfloat16
    wdt = mybir.dt.float8e5

    nc = bacc.Bacc("TRN2", target_bir_lowering=False, debug=False,
                   num_devices=N_CORES)

    xTb_d = nc.declare_dram_parameter("xTb", [128, 8, 128], bf16, isOutput=False)
    comb_d = nc.declare_dram_parameter("comb", [128, 2], f32, isOutput=False)
    wg_d = [nc.declare_dram_parameter(f"wg{e}", [128, 8, 4096], wdt, isOutput=False)
            for e in range(EXP_PER_CORE)]
    wd_d = [nc.declare_dram_parameter(f"wd{e}", [128, 16, 1024], wdt, isOutput=False)
            for e in range(EXP_PER_CORE)]
    bgu_d = [nc.declare_dram_parameter(f"bgu{e}", [1, 4096], bf16, isOutput=False)
             for e in range(EXP_PER_CORE)]
    out_d = nc.declare_dram_parameter("out", [128, 1024], f32, isOutput=True)
    dbg_d = nc.declare_dram_parameter("dbg", [128, 128], f32, isOutput=True)

    AF = mybir.ActivationFunctionType
    OP = mybir.AluOpType

    hw_engines = [None, None]  # filled with (sync, scalar) below

    with tile.TileContext(nc) as tc:
        with (
            tc.tile_pool(name="const", bufs=1) as constp,
            tc.tile_pool(name="wg", bufs=8) as wgp,
            tc.tile_pool(name="wd", bufs=4) as wdp,
            tc.tile_pool(name="hp", bufs=2) as hp,
            tc.tile_pool(name="silp", bufs=2) as silp,
            tc.tile_pool(name="htp", bufs=2) as htp,
            tc.tile_pool(name="ytp", bufs=2) as ytp,
            tc.tile_pool(name="psgu", bufs=4, space="PSUM") as ps_gu,
            tc.tile_pool(name="psy", bufs=2, space="PSUM") as ps_yp,
            tc.tile_pool(name="pst", bufs=2, space="PSUM") as ps_tp,
        ):
            # Weight streams ride Sync (HWDGE) + GpSimd (SWDGE) so the
            # Scalar engine's FIFO stays free for the silu activations.
            dma_engines = [nc.sync, nc.gpsimd]
            _dma_i = [0]

            def dma(out, in_):
                eng = dma_engines[_dma_i[0] % 2]
                _dma_i[0] += 1
                eng.dma_start(out=out, in_=in_)

            # ---- small constants (HWDGE, land fast) ----
            xT = constp.tile([128, 8, 128], bf16)
            nc.scalar.dma_start(out=xT[:], in_=xTb_d[:])
            combine = constp.tile([128, 2], f32)
            nc.scalar.dma_start(out=combine[:], in_=comb_d[:])
            bgu_t = []
            for e in range(EXP_PER_CORE):
                bg = constp.tile([1, 4096], bf16, tag=f"bgu{e}")
                nc.scalar.dma_start(out=bg[:], in_=bgu_d[e][:])
                bgu_t.append(bg)

            ones_t = constp.tile([1, 128], bf16)
            nc.vector.memset(ones_t[:], 1.0)
            ident = constp.tile([128, 128], f32)
            nc.vector.memset(ident[:], 1.0)
            nc.gpsimd.affine_select(
                out=ident[:], in_=ident[:],
                compare_op=OP.is_equal, fill=0.0, base=0,
                pattern=[[-1, 128]], channel_multiplier=1,
            )

            # ---- all weight DMAs issued up front; everything resident ----
            # gu chunks: [128, 4, 2048] (1 MB): (column half, kp-quad)
            wg_t = [[None] * 4 for _ in range(EXP_PER_CORE)]
            wd_t = [[None] * 2 for _ in range(EXP_PER_CORE)]
            for e in range(EXP_PER_CORE):
                for half in range(2):
                    for ci in range(2):
                        t = wgp.tile([128, 4, 2048], wdt, tag="wg")
                        dma(t[:], wg_d[e][:, 4 * ci:4 * ci + 4,
                                          half * 2048:(half + 1) * 2048])
                        wg_t[e][half * 2 + ci] = t
                for ci in range(2):
                    t = wdp.tile([128, 8, 1024], wdt, tag="wd")
                    dma(t[:], wd_d[e][:, 8 * ci:8 * ci + 8, :])
                    wd_t[e][ci] = t

            # ---- PE warmup: dummy matmuls (on memset data, so they have
            # no DMA dependency) so HAM is at K=8/8 when the first weight
            # chunk lands. Results funneled to dbg so DCE can't drop them.
            warm_w = constp.tile([128, 512], bf16, tag="warmw")
            nc.vector.memset(warm_w[:], 0.001)
            ps_w = ps_gu.tile([128, 512], f32, tag="psgu")
            for i in range(N_WARM):
                nc.tensor.matmul(ps_w[:], warm_w[:, :128], warm_w[:],
                                 start=True, stop=True)
            warm_sb = constp.tile([128, 128], f32, tag="warm")
            nc.scalar.activation(warm_sb[:], ps_w[:, :128], AF.Copy)
            nc.sync.dma_start(out=dbg_d[:], in_=warm_sb[:])

            acc = constp.tile([128, 1024], f32)

            # ---- experts ----
            for e in range(EXP_PER_CORE):
                h_sb = hp.tile([128, 2048], bf16, tag="h")
                hT32 = htp.tile([128, 8, 128], f32, tag="hT")
                for oc in range(4):
                    half, sub = oc // 2, oc % 2
                    ps_g = ps_gu.tile([128, 512], f32, tag="psgu")
                    ps_u = ps_gu.tile([128, 512], f32, tag="psgu")
                    nc.tensor.matmul(
                        ps_g[:], ones_t[:],
                        bgu_t[e][:, oc * 1024:oc * 1024 + 512],
                        start=True, stop=False)
                    nc.tensor.matmul(
                        ps_u[:], ones_t[:],
                        bgu_t[e][:, oc * 1024 + 512:oc * 1024 + 1024],
                        start=True, stop=False)
                    for k in range(8):
                        ch = wg_t[e][half * 2 + k // 4]
                        stat = xT[:, k, :]
                        nc.tensor.matmul(
                            ps_g[:], stat,
                            ch[:, k % 4, sub * 1024:sub * 1024 + 512],
                            start=False, stop=(k == 7))
                        nc.tensor.matmul(
                            ps_u[:], stat,
                            ch[:, k % 4, sub * 1024 + 512:sub * 1024 + 1024],
                            start=False, stop=(k == 7))
                    sil = silp.tile([128, 512], f32, tag="sil")
                    nc.scalar.activation(sil[:], ps_g[:], AF.Silu)
                    nc.vector.tensor_tensor(
                        h_sb[:, oc * 512:(oc + 1) * 512], sil[:], ps_u[:],
                        op=OP.mult)
                    # fp32-viewed transposes of the finished 512 columns
                    for kt in (2 * oc, 2 * oc + 1):
                        ps_t = ps_tp.tile([128, 128], f32, tag="pst")
                        nc.tensor.transpose(
                            ps_t[:],
                            h_sb[:, 256 * kt:256 * (kt + 1)].bitcast(f32),
                            ident[:])
                        nc.vector.tensor_copy(hT32[:, kt, :], ps_t[:])

                # down projection: h.T (bf16 pairs in fp32 lanes) x W_d
                ps_y0 = ps_yp.tile([128, 512], f32, tag="psy")
                ps_y1 = ps_yp.tile([128, 512], f32, tag="psy")
                ps_y = [ps_y0, ps_y1]
                hT16 = hT32[:, :, :].bitcast(bf16)  # [128, 8, 256]
                for kt in range(8):
                    for o in range(2):
                        stat = hT16[:, kt, o::2]
                        jg = 2 * kt + o
                        ch = wd_t[e][jg // 8]
                        j = jg % 8
                        for c in range(2):
                            nc.tensor.matmul(
                                ps_y[c][:], stat,
                                ch[:, j, c * 512:(c + 1) * 512],
                                start=(kt == 0 and o == 0),
                                stop=(kt == 7 and o == 1))
                ce = combine[:, e:e + 1]
                for c in range(2):
                    if e == 0:
                        nc.vector.tensor_scalar(
                            acc[:, c * 512:(c + 1) * 512], ps_y[c][:],
                            ce, None, op0=OP.mult)
                    else:
                        ytmp = ytp.tile([128, 512], f32, tag="ytmp")
                        nc.vector.tensor_scalar(ytmp[:], ps_y[c][:],
                                                ce, None, op0=OP.mult)
                        nc.vector.tensor_tensor(
                            acc[:, c * 512:(c + 1) * 512],
                            acc[:, c * 512:(c + 1) * 512], ytmp[:],
                            op=OP.add)
                        nc.sync.dma_start(
                            out=out_d[:, c * 512:(c + 1) * 512],
                            in_=acc[:, c * 512:(c + 1) * 512])

    nc.finalize()
    return nc


# column permutation: [g0 u0 g1 u1 g2 u2 g3 u3] (512 each)
_COLPERM = np.concatenate(
    [np.r_[oc * 512:(oc + 1) * 512, 2048 + oc * 512:2048 + (oc + 1) * 512]
     for oc in range(4)])

# down f interleave: row j of 16, lane i: f = 256*(j//2) + 2*i + (j%2)
_J = np.arange(16)
_I = np.arange(128)
_DOWN_F = (256 * (_J[:, None] // 2) + 2 * _I[None, :] + (_J[:, None] % 2))  # [16,128]


def _prep_inputs(hidden_states, router_w, bias_gu, bias_down,
                 blocks_gu, scales_gu, blocks_down, scales_down):
    x = np.asarray(hidden_states, dtype=np.float32).reshape(T, H)
    xT = np.ascontiguousarray(x.T)                         # [1024, 128]
    xTb = np.ascontiguousarray(
        xT.reshape(8, 128, 128).transpose(1, 0, 2)).astype(BF16)

    # host router: logits -> top-2 -> softmax -> dense combine [T, E]
    logits = x @ np.asarray(router_w, dtype=np.float32).T
    order = np.argsort(-logits, axis=-1, kind="stable")
    i1, i2 = order[:, 0], order[:, 1]
    v1 = logits[np.arange(T), i1]
    v2 = logits[np.arange(T), i2]
    w1 = 1.0 / (1.0 + np.exp(v2 - v1))
    w2 = 1.0 - w1
    combine = np.zeros((T, E), dtype=np.float32)
    combine[np.arange(T), i1] = w1
    combine[np.arange(T), i2] = w2

    bias_down_f = np.asarray(bias_down, dtype=np.float32)
    host_bias = combine @ bias_down_f                       # [T, H]

    w_gu = _dequant(np.asarray(blocks_gu), np.asarray(scales_gu))      # [E,4096,1024]
    w_dn = _dequant(np.asarray(blocks_down), np.asarray(scales_down))  # [E,1024,2048]
    bias_gu_f = np.asarray(bias_gu, dtype=np.float32)

    in_maps = []
    for core in range(N_CORES):
        my = [core * EXP_PER_CORE + j for j in range(EXP_PER_CORE)]
        m = {"xTb": xTb,
             "comb": np.ascontiguousarray(combine[:, my]).astype(np.float32)}
        for j, ge in enumerate(my):
            wre = w_gu[ge][_COLPERM]                       # [4096, 1024]
            wT = np.ascontiguousarray(wre.T)               # [1024, 4096]
            m[f"wg{j}"] = np.ascontiguousarray(
                wT.reshape(8, 128, 4096).transpose(1, 0, 2)).astype(E5M2)
            # down: wd[i, j2, c] = W_d[c, f(j2, i)]
            wd = w_dn[ge][:, _DOWN_F]                      # [1024, 16, 128]
            m[f"wd{j}"] = np.ascontiguousarray(
                wd.transpose(2, 1, 0)).astype(E5M2)        # [128, 16, 1024]
            m[f"bgu{j}"] = bias_gu_f[ge][_COLPERM].reshape(1, 4096).astype(BF16)
        in_maps.append(m)
    return in_maps, host_bias


def kernel(hidden_states, router_w, bias_gu, bias_down,
           blocks_gu, scales_gu, blocks_down, scales_down, _trace=False):
    from concourse.bass_utils import run_bass_kernel_spmd

    if "nc" not in _compiled:
        _compiled["nc"] = _build()
    nc = _compiled["nc"]

    in_maps, host_bias = _prep_inputs(
        hidden_states, router_w, bias_gu, bias_down,
        blocks_gu, scales_gu, blocks_down, scales_down)
    res = run_bass_kernel_spmd(nc, in_maps, list(range(N_CORES)), trace=_trace)
    total = host_bias.copy()
    for om in res.results:
        total += np.asarray(om["out"], dtype=np.float32)
    out = total.reshape(1, T, H)
    if _trace:
        return out, res
    return out
